# revision 5
# baseline (speedup 1.0000x reference)
"""KPConv (nn_KPConvFPN) Trainium2 Bass kernel, v2.

Sharding: 8 cores; core c handles batch b=c//2, query half (c%2)*8192.
Changes vs v1:
  - Combined gather table is built on HOST (numpy) and shipped as input:
    row m (256B) = [64 x fp16 feat | f32 sx,sy,sz at f32-cols 32..34 |
    fp16 z at col 70], z = (sum_c |f| > 0).
  - All matmuls fp16 (weights, kbd, wtt, count) -> 1 cy/row on PE.
  - einsum2 output [o=128, n=512] divided/biased in place and stored to a
    TRANSPOSED dram output [C_OUT, NQ]; host transposes back (no PE
    transposes, no trsb copies).
  - count path: replicate-then-reciprocal on [128,512] (fast) instead of
    reciprocal on [1,512].
  - One idx DMA per kw-group instead of 16 small ones.
"""
import json
import math
import os

SKIP = set()

import numpy as np
import jax

import concourse.bass as bass
import concourse.mybir as mybir
from concourse.tile import TileContext
from concourse import library_config
from concourse import bass2jax

F32 = mybir.dt.float32
F16 = mybir.dt.float16
I16 = mybir.dt.int16

B, N, M, K = 4, 16384, 16384, 16
C_IN, C_OUT, P = 64, 128, 15
SIGMA = 0.03
N_CORES = 8
NQ_CORE = N // 2            # 8192 queries per core
NK_CORE = NQ_CORE * K       # 131072 gathered rows per core
ST_Q = 512                  # queries per supertile
N_ST = NQ_CORE // ST_Q      # 16
KW_ST = 2                   # supertiles per kw group
G_ST = ST_Q * K // 128      # 64 g-cols per supertile
ROW16 = 128                 # fp16 units per table row (256B)
GCHUNK = 1024               # idx per dma_gather call

# ---------------------------------------------------------------------------
# walrus workaround: this nix walrus build supports ONE sync-wait per
# instruction; split extra waits onto NoOps inserted before the offender
# (same-engine program order preserves semantics). Also run
# codegen_inst_isa_subclasses (Bacc does; raw Bass doesn't) so extended
# instructions get their ISA bytes.
_orig_to_json_bytes = bass.Bass.to_json_bytes


def _fix_block(bb, ctr):
    insts = bb.get("instructions")
    if not isinstance(insts, list):
        return
    new = []
    for inst in insts:
        si = inst.get("sync_info")
        ow = si.get("on_wait") if isinstance(si, dict) else None
        if ow and len(ow) > 1:
            for w in ow[:-1]:
                ctr[0] += 1
                nop = {"engine": inst["engine"], "ins": [], "outs": [],
                       "name": f"I-wsplit-{ctr[0]}", "opcode": "NoOp",
                       "sync_info": {"on_update": [], "on_wait": [w]},
                       "text_hint": "wsplit"}
                if "debug" in inst:
                    nop["debug"] = inst["debug"]
                new.append(nop)
            si["on_wait"] = [ow[-1]]
        new.append(inst)
    bb["instructions"] = new


def _walk(o, ctr):
    if isinstance(o, dict):
        if isinstance(o.get("instructions"), list):
            _fix_block(o, ctr)
        for v in o.values():
            _walk(v, ctr)
    elif isinstance(o, list):
        for v in o:
            _walk(v, ctr)


def _to_json_bytes_split(self):
    mybir.codegen_inst_isa_subclasses(self)
    raw = _orig_to_json_bytes(self)
    d = json.loads(raw)
    ctr = [0]
    _walk(d, ctr)
    return json.dumps(d).encode()


bass.Bass.to_json_bytes = _to_json_bytes_split


def ap_view(t_ap, extra_offset, dims):
    """AP over tile t_ap with explicit free dims [[step, count], ...]
    (steps in elements); partition dim is taken from the tile."""
    return bass.AP(t_ap.tensor, t_ap.offset + extra_offset,
                   [t_ap.ap[0]] + list(dims))


DEBUG = False


def build_bass(kp, skip=()):
    global SKIP
    SKIP = set(skip)
    """kp: (15, 3) float32 numpy kernel points (runtime values baked)."""
    nc = bass.Bass(dynamic_dma_scratch_size=32768, num_swdge_queues=4)

    table_in = nc.dram_tensor("table", [M, ROW16], F16, kind="ExternalInput")
    qrep_in = nc.dram_tensor("qrep", [128, NK_CORE // 128, 3], F32,
                             kind="ExternalInput")
    idx_in = nc.dram_tensor("idx", [128, NK_CORE // 16], I16,
                            kind="ExternalInput")
    w_in = nc.dram_tensor("w", [C_IN, P * C_OUT], F16, kind="ExternalInput")
    bias_in = nc.dram_tensor("bias", [C_OUT, 1], F32, kind="ExternalInput")
    mask120_in = nc.dram_tensor("mask120", [128, 120], F16, kind="ExternalInput")
    mask16_in = nc.dram_tensor("mask16", [128, 8], F16, kind="ExternalInput")
    ones1_in = nc.dram_tensor("ones1", [1, 128], F16, kind="ExternalInput")
    kpb_in = nc.dram_tensor("kpb", [128, 48], F32, kind="ExternalInput")
    kpallf_in = nc.dram_tensor("kpallf", [128, 45], F16, kind="ExternalInput")
    onesc_in = nc.dram_tensor("onesc", [128, 1], F16, kind="ExternalInput")
    out_t = nc.dram_tensor("out", [C_OUT, NQ_CORE], F32, kind="ExternalOutput")
    dbg_t = (nc.dram_tensor("dbg", [128, 4096], F32, kind="ExternalOutput")
             if DEBUG else None)

    # library load as raw preamble (before Tile scheduling) so it is
    # guaranteed to precede every dma_gather on the Pool engine.
    nc.gpsimd.load_library(library_config.mlp)

    with TileContext(nc) as tc:
        with tc.tile_pool(name="const", bufs=1) as cpool, \
             tc.tile_pool(name="gath", bufs=2) as gpool, \
             tc.tile_pool(name="kwp", bufs=2) as kwpool, \
             tc.tile_pool(name="kbd", bufs=1) as kbpool, \
             tc.tile_pool(name="wt", bufs=2) as wtpool, \
             tc.tile_pool(name="sm", bufs=3) as smpool, \
             tc.tile_pool(name="fin", bufs=2) as fpool, \
             tc.tile_pool(name="ps1", bufs=4, space="PSUM") as ps1pool, \
             tc.tile_pool(name="ps2", bufs=2, space="PSUM") as ps2pool, \
             tc.tile_pool(name="ps3", bufs=1, space="PSUM") as ps3pool:

            # ---- constants ----
            wp_t = cpool.tile([C_IN, P * C_OUT], F16, tag="wp")
            nc.sync.dma_start(wp_t[:], w_in[:])
            bias_t = cpool.tile([C_OUT, 1], F32, tag="bias")
            nc.sync.dma_start(bias_t[:], bias_in[:])
            mask120_t = cpool.tile([128, 120], F16, tag="m120")
            nc.sync.dma_start(mask120_t[:], mask120_in[:])
            mask16_t = cpool.tile([128, 8], F16, tag="m16")
            nc.sync.dma_start(mask16_t[:], mask16_in[:])
            ones1_t = cpool.tile([1, 128], F16, tag="ones1")
            nc.sync.dma_start(ones1_t[:], ones1_in[:])
            kpb_t = cpool.tile([128, 48], F32, tag="kpb")
            nc.sync.dma_start(kpb_t[:], kpb_in[:])
            kpallf_t = cpool.tile([128, 45], F16, tag="kpallf")
            nc.sync.dma_start(kpallf_t[:], kpallf_in[:])
            onesc_t = cpool.tile([128, 1], F16, tag="onesc")
            nc.sync.dma_start(onesc_t[:], onesc_in[:])
            nidx_reg = nc.gpsimd.to_reg(GCHUNK)

            _main_pipeline(nc, tc, gpool, kwpool, kbpool, wtpool, smpool,
                           fpool, ps1pool, ps2pool, ps3pool, kp,
                           qrep_in, idx_in, out_t, table_in, wp_t, bias_t,
                           mask120_t, mask16_t, ones1_t, kpb_t,
                           onesc_t, nidx_reg, kpallf_t, dbg_t)
    return nc


def _main_pipeline(nc, tc, gpool, kwpool, kbpool, wtpool, smpool, fpool,
                   ps1pool, ps2pool, ps3pool, kp, qrep_in, idx_in, out_t,
                   table, wp_t, bias_t, mask120_t, mask16_t, ones1_t, kpb_t,
                   onesc_t, nidx_reg, kpallf_t=None, dbg_t=None):
    for kg in range(N_ST // KW_ST):  # kw group of 2 supertiles
        GQ = KW_ST * ST_Q            # 1024 queries
        GG = KW_ST * G_ST            # 128 g-cols
        NIDX_G = GQ * K              # 16384 idx per group
        gt = gpool.tile([128, GG, ROW16], F16, tag="gath")
        gt32 = gt[:].bitcast(F32)  # [128, GG, 64] f32 view
        # one idx load for the whole group
        idxg = smpool.tile([128, NIDX_G // 16], I16, tag="idxg")
        nc.sync.dma_start(
            idxg[:], idx_in[:, kg * (NIDX_G // 16):(kg + 1) * (NIDX_G // 16)])
        if "gather" in SKIP:
            nc.vector.memset(gt[:], 0.0)
        ncalls = NIDX_G // GCHUNK
        gcols = GG // ncalls
        for g in range(ncalls if "gather" not in SKIP else 0):
            nc.gpsimd.dma_gather(
                gt[:, g * gcols:(g + 1) * gcols, :], table[:],
                idxg[:, g * (GCHUNK // 16):(g + 1) * (GCHUNK // 16)],
                GCHUNK, nidx_reg, ROW16, queue_num=g % 4)
        # qrep slice
        qr = smpool.tile([128, GG, 3], F32, tag="qr")
        nc.sync.dma_start(qr[:], qrep_in[:, kg * GG:(kg + 1) * GG, :])
        # rel = s - q (fp16 out: costs ~1e-2 rel err end-to-end, gate 2e-2)
        rel = smpool.tile([128, GG, 3], F16, tag="rel")
        nc.vector.tensor_tensor(
            out=rel[:],
            in0=ap_view(gt32, 32, [[64, GG], [1, 3]]),
            in1=qr[:], op=mybir.AluOpType.subtract)
        # d2 batched: dall[g,p,d] = rel[g,d] - kp[p,d] (one TT), square
        # in place (one ACT), reduce over d (one DVE reduce) -> fp16 d2
        kwt16 = kwpool.tile([128, GG, P], F16, tag="kw16")
        dall = kwpool.tile([128, GG * P * 3], F16, tag="dall")
        if "kw" in SKIP:
            nc.vector.memset(kwt16[:], 0.0)
        if "kw" not in SKIP:
            nc.vector.tensor_tensor(
                out=dall[:],
                in0=ap_view(rel[:], 0, [[3, GG], [0, P], [1, 3]]),
                in1=ap_view(kpallf_t[:], 0, [[0, GG], [3, P], [1, 3]]),
                op=mybir.AluOpType.subtract)
            nc.scalar.activation(dall[:], dall[:],
                                 mybir.ActivationFunctionType.Square,
                                 bias=0.0, scale=1.0)
            # sum of 3 squares as two strided TT adds (tensor_reduce is
            # ~2x slower for this shape)
            nc.vector.tensor_tensor(
                out=ap_view(kwt16[:], 0, [[1, GG * P], [1, 1]]),
                in0=ap_view(dall[:], 0, [[3, GG * P], [1, 1]]),
                in1=ap_view(dall[:], 1, [[3, GG * P], [1, 1]]),
                op=mybir.AluOpType.add)
            nc.vector.tensor_tensor(
                out=ap_view(kwt16[:], 0, [[1, GG * P], [1, 1]]),
                in0=ap_view(kwt16[:], 0, [[1, GG * P], [1, 1]]),
                in1=ap_view(dall[:], 2, [[3, GG * P], [1, 1]]),
                op=mybir.AluOpType.add)
            # kw = relu(1 - sqrt(d2)/sigma), fp16 in place
            nc.scalar.activation(kwt16[:], kwt16[:],
                                 mybir.ActivationFunctionType.Sqrt,
                                 bias=0.0, scale=1.0)
            nc.scalar.activation(kwt16[:], kwt16[:],
                                 mybir.ActivationFunctionType.Relu,
                                 bias=1.0, scale=kpb_t[:, 46:47])
        if dbg_t is not None and kg == 6:
            stg = fpool.tile([128, 384], F32, tag="dbgstg")
            # z col per g
            nc.vector.tensor_copy(
                stg[:, 0:128],
                ap_view(gt[:], 70, [[ROW16, GG], [1, 1]]))
            # feat col 0 per g
            nc.vector.tensor_copy(
                stg[:, 128:256],
                ap_view(gt[:], 0, [[ROW16, GG], [1, 1]]))
            # kw p=0 per g
            nc.vector.tensor_copy(
                stg[:, 256:384],
                ap_view(kwt16[:], 0, [[P, GG], [1, 1]]))
            nc.sync.dma_start(dbg_t[:, 0:384], stg[:])

        for sti in range(KW_ST):
            st = kg * KW_ST + sti
            # kwbd (2 half-ST TT ops): [128, (bl32, q8, p15)] fp16
            kbd = kbpool.tile([128, 3840], F16, tag="kbd")
            kbd2 = kbpool.tile([128, 3840], F16, tag="kbd2")
            if "kwbd" in SKIP:
                nc.vector.memset(kbd[:], 0.0)
                nc.vector.memset(kbd2[:], 0.0)
            for hf, kb in ((0, kbd), (1, kbd2)) if "kwbd" not in SKIP else ():
                bl0 = sti * G_ST + hf * 32
                # p-major block layout: col (bl, p, q) so einsum2 rhs slices
                # are contiguous runs of 8
                nc.vector.tensor_tensor(
                    out=ap_view(kb[:], 0, [[120, 32], [8, 15], [1, 8]]),
                    in0=ap_view(kwt16[:], bl0 * P, [[P, 32], [1, P], [0, 8]]),
                    in1=ap_view(mask120_t[:], 0, [[0, 32], [8, 15], [1, 8]]),
                    op=mybir.AluOpType.mult)
            # einsum1: 64 blocks -> wtt fp16
            wtt = wtpool.tile([64, 7680], F16, tag="wt")
            if "e1" in SKIP:
                nc.vector.memset(wtt[:], 0.0)
            for bg in range(16 if "e1" not in SKIP else 0):
                pse1 = ps1pool.tile([64, 480], F32, tag="pse1")
                for j in range(4):
                    bl = bg * 4 + j          # block in supertile
                    blg = sti * G_ST + bl    # g-col in group tile
                    kb = kbd if bl < 32 else kbd2
                    kbl = bl % 32
                    nc.tensor.matmul(
                        pse1[:, j * 120:(j + 1) * 120],
                        ap_view(gt[:], blg * ROW16, [[1, C_IN]]),
                        ap_view(kb[:], kbl * 120, [[1, 120]]),
                        start=True, stop=True)
                # evict (split DVE/ACT), f32 -> fp16
                nc.vector.tensor_copy(
                    wtt[:, bg * 480:bg * 480 + 240], pse1[:, 0:240])
                nc.scalar.activation(
                    wtt[:, bg * 480 + 240:bg * 480 + 480], pse1[:, 240:480],
                    mybir.ActivationFunctionType.Copy, bias=0.0, scale=1.0)
            # count: zbd = z * mask16 (fp16) -> ones-col matmul -> replicate
            zbd = smpool.tile([128, 512], F16, tag="zbd")
            nc.vector.tensor_tensor(
                out=zbd[:].rearrange("a (g j q) -> a g j q", g=16, j=4),
                in0=ap_view(gt[:], (sti * G_ST) * ROW16 + 70,
                            [[512, 16], [128, 4], [0, 8]]),
                in1=ap_view(mask16_t[:], 0, [[0, 16], [0, 4], [1, 8]]),
                op=mybir.AluOpType.mult)
            pscnt = ps3pool.tile([1, 512], F32, tag="pscnt")
            nc.tensor.matmul(pscnt[:], onesc_t[:], zbd[:],
                             start=True, stop=True)
            cntrow = smpool.tile([1, 512], F16, tag="cntrow")
            nc.scalar.copy(cntrow[:], pscnt[:])
            psrep = ps3pool.tile([128, 512], F32, tag="psrep")
            nc.tensor.matmul(psrep[:], ones1_t[:], cntrow[:],
                             start=True, stop=True)
            cntinv = smpool.tile([128, 512], F32, tag="cntinv")
            nc.vector.tensor_scalar(out=cntinv[:], in0=psrep[:],
                                    scalar1=1.0, scalar2=None,
                                    op0=mybir.AluOpType.max)
            # 1/x = exp(-ln(x)); x is an integer count in [1, 16]
            nc.scalar.activation(cntinv[:], cntinv[:],
                                 mybir.ActivationFunctionType.Ln,
                                 bias=0.0, scale=1.0)
            nc.scalar.activation(cntinv[:], cntinv[:],
                                 mybir.ActivationFunctionType.Exp,
                                 bias=0.0, scale=-1.0)

            # einsum2: out[o, n] accumulated over p (fp16 operands)
            pse2 = ps2pool.tile([128, 512], F32, tag="pse2")
            for p in range(P if "e2" not in SKIP else 1):
                nc.tensor.matmul(
                    pse2[:],
                    ap_view(wp_t[:], p * C_OUT, [[1, C_OUT]]),
                    ap_view(wtt[:], p * 8, [[480, 16], [120, 4], [1, 8]]),
                    start=(p == 0), stop=True)
            if dbg_t is not None and st == 12:
                stg2 = fpool.tile([128, 1536], F32, tag="dbgstg2")
                nc.vector.tensor_copy(stg2[:, 0:512], cntinv[:])
                nc.vector.tensor_copy(stg2[:, 512:1024], pse2[:])
                nc.vector.tensor_copy(stg2[:, 1024:1536], zbd[:])
                nc.sync.dma_start(dbg_t[:, 512:2048], stg2[:])
                stg3 = fpool.tile([64, 512], F32, tag="dbgstg3")
                nc.vector.tensor_copy(stg3[:], wtt[:, 0:512])
                nc.sync.dma_start(dbg_t[0:64, 2048:2560], stg3[:])
            # divide by count, add bias, store transposed
            e2sb = fpool.tile([128, 512], F32, tag="e2sb")
            nc.vector.tensor_tensor(out=e2sb[:], in0=pse2[:], in1=cntinv[:],
                                    op=mybir.AluOpType.mult)
            nc.vector.tensor_scalar(out=e2sb[:], in0=e2sb[:],
                                    scalar1=bias_t[:], scalar2=None,
                                    op0=mybir.AluOpType.add)
            nc.sync.dma_start(out_t[:, st * 512:(st + 1) * 512], e2sb[:])


def _make_runner(nc, n_cores):
    bass2jax.install_neuronx_cc_hook()
    from jax.sharding import Mesh, PartitionSpec
    from jax.experimental.shard_map import shard_map

    partition_name = nc.partition_id_tensor.name if nc.partition_id_tensor else None
    in_names, out_names, out_avals, zero_outs = [], [], [], []
    for alloc in nc.m.functions[0].allocations:
        if not isinstance(alloc, mybir.MemoryLocationSet):
            continue
        name = alloc.memorylocations[0].name
        if alloc.kind == "ExternalInput":
            if name != partition_name:
                in_names.append(name)
        elif alloc.kind == "ExternalOutput":
            shape = tuple(alloc.tensor_shape)
            dtype = mybir.dt.np(alloc.dtype)
            out_names.append(name)
            out_avals.append(jax.core.ShapedArray(shape, dtype))
            zero_outs.append(np.zeros(shape, dtype))
    n_params = len(in_names)
    n_outs = len(out_avals)
    all_in = in_names + out_names + ([partition_name] if partition_name else [])

    def _body(*args):
        operands = list(args)
        if partition_name is not None:
            operands.append(bass2jax.partition_id_tensor())
        outs = bass2jax._bass_exec_p.bind(
            *operands, out_avals=tuple(out_avals), in_names=tuple(all_in),
            out_names=tuple(out_names), lowering_input_output_aliases=(),
            sim_require_finite=False, sim_require_nnan=False, nc=nc)
        return tuple(outs)

    devices = jax.devices()[:n_cores]
    mesh = Mesh(np.asarray(devices), ("core",))
    in_specs = (PartitionSpec("core"),) * (n_params + n_outs)
    out_specs = (PartitionSpec("core"),) * n_outs
    jit_fn = jax.jit(
        shard_map(_body, mesh=mesh, in_specs=in_specs, out_specs=out_specs,
                  check_rep=False), keep_unused=True)

    def run(in_maps):
        per_core = [[np.asarray(m[n]) for n in in_names] for m in in_maps]
        args = [np.concatenate([per_core[c][i] for c in range(n_cores)], axis=0)
                for i in range(n_params)]
        args += [np.zeros((n_cores * z.shape[0], *z.shape[1:]), z.dtype)
                 for z in zero_outs]
        outs = [np.asarray(o) for o in jit_fn(*args)]
        return [{n: outs[i].reshape(n_cores, *out_avals[i].shape)[c]
                 for i, n in enumerate(out_names)}
                for c in range(n_cores)], jit_fn, args

    return run


_BUILT = {}
_NCS = {}


def _get_runner(kp):
    key = kp.tobytes()
    if key not in _BUILT:
        nc = build_bass(kp)
        _NCS[key] = nc
        _BUILT[key] = _make_runner(nc, N_CORES)
    return _BUILT[key]


def _host_prep(query_points, support_points, support_features,
               neighbor_indices, weights, bias, kernel_points):
    qp = np.asarray(query_points, np.float32)
    sp = np.asarray(support_points, np.float32)
    sf = np.asarray(support_features, np.float32)
    ni = np.asarray(neighbor_indices)
    ni = np.clip(ni, 0, M - 1).astype(np.int16)
    w = np.ascontiguousarray(
        np.asarray(weights, np.float32).transpose(1, 0, 2).reshape(
            C_IN, P * C_OUT)).astype(np.float16)
    bias = np.asarray(bias, np.float32).reshape(C_OUT, 1)

    # host-built gather tables, one per batch
    tables = []
    for b in range(B):
        tbl = np.zeros((M, ROW16), np.float16)
        tbl[:, 0:C_IN] = sf[b].astype(np.float16)
        tblf = tbl.view(np.float32)
        tblf[:, 32:35] = sp[b]
        z = (np.abs(sf[b]).sum(axis=1) > 0).astype(np.float16)
        tbl[:, 70] = z
        tables.append(tbl)

    # p-major: col (p, q) -> 1 iff partition//16 == q
    mask120 = np.zeros((128, 120), np.float16)
    for q in range(8):
        for p in range(15):
            mask120[q * 16:(q + 1) * 16, p * 8 + q] = 1.0
    mask16 = np.zeros((128, 8), np.float16)
    for q in range(8):
        mask16[q * 16:(q + 1) * 16, q] = 1.0
    ones1 = np.ones((1, 128), np.float16)
    kpv = np.asarray(kernel_points, np.float32)
    kpb = np.zeros((128, 48), np.float32)
    for p in range(P):
        for d in range(3):
            kpb[:, 3 * p + d] = -kpv[p, d]
    kpb[:, 45] = 1e-10
    kpb[:, 46] = -1.0 / SIGMA
    kpallf = np.tile(kpv.reshape(1, 45), (128, 1)).astype(np.float16)

    in_maps = []
    for c in range(N_CORES):
        b, half = divmod(c, 2)
        n0 = half * NQ_CORE
        idx = ni[b, n0:n0 + NQ_CORE, :].reshape(NK_CORE)
        # chunk order: idx j in chunk -> partition j%16 (k), col j//16;
        # stream order is already (query, k) = natural
        idx_l = idx.reshape(NK_CORE // 16, 16).T          # [16, NK/16]
        idx_l = np.tile(idx_l, (8, 1))                    # [128, NK/16]
        qrep = np.repeat(qp[b, n0:n0 + NQ_CORE, :], K, axis=0)  # [NK, 3]
        qrep = qrep.reshape(NK_CORE // 128, 128, 3).transpose(1, 0, 2)
        qrep = np.ascontiguousarray(qrep)
        in_maps.append({
            "table": tables[b], "qrep": qrep,
            "idx": np.ascontiguousarray(idx_l),
            "w": w, "bias": bias, "mask120": mask120, "mask16": mask16,
            "ones1": ones1, "kpb": kpb, "kpallf": kpallf,
            "onesc": np.ones((128, 1), np.float16),
        })
    return in_maps


def kernel(query_points, support_points, support_features, neighbor_indices,
           weights, bias, kernel_points):
    kp = np.asarray(kernel_points, np.float32)
    run = _get_runner(kp)
    in_maps = _host_prep(query_points, support_points, support_features,
                         neighbor_indices, weights, bias, kernel_points)
    results, _, _ = run(in_maps)
    out = np.zeros((B, N, C_OUT), np.float32)
    for c in range(N_CORES):
        b, half = divmod(c, 2)
        n0 = half * NQ_CORE
        out[b, n0:n0 + NQ_CORE, :] = results[c]["out"].T
    return out


# revision 6
# speedup vs baseline: 1.2315x; 1.2315x over previous
"""KPConv (nn_KPConvFPN) Trainium2 Bass kernel, v2.

Sharding: 8 cores; core c handles batch b=c//2, query half (c%2)*8192.
Changes vs v1:
  - Combined gather table is built on HOST (numpy) and shipped as input:
    row m (256B) = [64 x fp16 feat | f32 sx,sy,sz at f32-cols 32..34 |
    fp16 z at col 70], z = (sum_c |f| > 0).
  - All matmuls fp16 (weights, kbd, wtt, count) -> 1 cy/row on PE.
  - einsum2 output [o=128, n=512] divided/biased in place and stored to a
    TRANSPOSED dram output [C_OUT, NQ]; host transposes back (no PE
    transposes, no trsb copies).
  - count path: replicate-then-reciprocal on [128,512] (fast) instead of
    reciprocal on [1,512].
  - One idx DMA per kw-group instead of 16 small ones.
"""
import json
import math
import os

SKIP = set()

import numpy as np
import jax

import concourse.bass as bass
import concourse.mybir as mybir
from concourse.tile import TileContext
from concourse import library_config
from concourse import bass2jax

F32 = mybir.dt.float32
F16 = mybir.dt.float16
I16 = mybir.dt.int16

B, N, M, K = 4, 16384, 16384, 16
C_IN, C_OUT, P = 64, 128, 15
SIGMA = 0.03
N_CORES = 8
NQ_CORE = N // 2            # 8192 queries per core
NK_CORE = NQ_CORE * K       # 131072 gathered rows per core
ST_Q = 512                  # queries per supertile
N_ST = NQ_CORE // ST_Q      # 16
KW_ST = 2                   # supertiles per kw group
G_ST = ST_Q * K // 128      # 64 g-cols per supertile
ROW16 = 128                 # fp16 units per table row (256B)
GCHUNK = 1024               # idx per dma_gather call

# ---------------------------------------------------------------------------
# walrus workaround: this nix walrus build supports ONE sync-wait per
# instruction; split extra waits onto NoOps inserted before the offender
# (same-engine program order preserves semantics). Also run
# codegen_inst_isa_subclasses (Bacc does; raw Bass doesn't) so extended
# instructions get their ISA bytes.
_orig_to_json_bytes = bass.Bass.to_json_bytes


def _fix_block(bb, ctr):
    insts = bb.get("instructions")
    if not isinstance(insts, list):
        return
    new = []
    for inst in insts:
        si = inst.get("sync_info")
        ow = si.get("on_wait") if isinstance(si, dict) else None
        if ow and len(ow) > 1:
            for w in ow[:-1]:
                ctr[0] += 1
                nop = {"engine": inst["engine"], "ins": [], "outs": [],
                       "name": f"I-wsplit-{ctr[0]}", "opcode": "NoOp",
                       "sync_info": {"on_update": [], "on_wait": [w]},
                       "text_hint": "wsplit"}
                if "debug" in inst:
                    nop["debug"] = inst["debug"]
                new.append(nop)
            si["on_wait"] = [ow[-1]]
        new.append(inst)
    bb["instructions"] = new


def _walk(o, ctr):
    if isinstance(o, dict):
        if isinstance(o.get("instructions"), list):
            _fix_block(o, ctr)
        for v in o.values():
            _walk(v, ctr)
    elif isinstance(o, list):
        for v in o:
            _walk(v, ctr)


def _to_json_bytes_split(self):
    mybir.codegen_inst_isa_subclasses(self)
    raw = _orig_to_json_bytes(self)
    d = json.loads(raw)
    ctr = [0]
    _walk(d, ctr)
    return json.dumps(d).encode()


bass.Bass.to_json_bytes = _to_json_bytes_split


def ap_view(t_ap, extra_offset, dims):
    """AP over tile t_ap with explicit free dims [[step, count], ...]
    (steps in elements); partition dim is taken from the tile."""
    return bass.AP(t_ap.tensor, t_ap.offset + extra_offset,
                   [t_ap.ap[0]] + list(dims))


DEBUG = False


def build_bass(kp, skip=()):
    global SKIP
    SKIP = set(skip)
    """kp: (15, 3) float32 numpy kernel points (runtime values baked)."""
    nc = bass.Bass(dynamic_dma_scratch_size=32768, num_swdge_queues=4)

    table_in = nc.dram_tensor("table", [M, ROW16], F16, kind="ExternalInput")
    qrep_in = nc.dram_tensor("qrep", [128, NK_CORE // 128, 3], F32,
                             kind="ExternalInput")
    idx_in = nc.dram_tensor("idx", [128, NK_CORE // 16], I16,
                            kind="ExternalInput")
    w_in = nc.dram_tensor("w", [C_IN, P * C_OUT], F16, kind="ExternalInput")
    bias_in = nc.dram_tensor("bias", [C_OUT, 1], F32, kind="ExternalInput")
    mask120_in = nc.dram_tensor("mask120", [128, 120], F16, kind="ExternalInput")
    mask16_in = nc.dram_tensor("mask16", [128, 8], F16, kind="ExternalInput")
    ones1_in = nc.dram_tensor("ones1", [1, 128], F16, kind="ExternalInput")
    kpb_in = nc.dram_tensor("kpb", [128, 48], F32, kind="ExternalInput")
    kpallf_in = nc.dram_tensor("kpallf", [128, 45], F16, kind="ExternalInput")
    onesc_in = nc.dram_tensor("onesc", [128, 1], F16, kind="ExternalInput")
    out_t = nc.dram_tensor("out", [C_OUT, NQ_CORE], F32, kind="ExternalOutput")
    dbg_t = (nc.dram_tensor("dbg", [128, 4096], F32, kind="ExternalOutput")
             if DEBUG else None)

    # library load as raw preamble (before Tile scheduling) so it is
    # guaranteed to precede every dma_gather on the Pool engine.
    nc.gpsimd.load_library(library_config.mlp)

    with TileContext(nc) as tc:
        with tc.tile_pool(name="const", bufs=1) as cpool, \
             tc.tile_pool(name="gath", bufs=2) as gpool, \
             tc.tile_pool(name="kwp", bufs=2) as kwpool, \
             tc.tile_pool(name="kbd", bufs=1) as kbpool, \
             tc.tile_pool(name="wt", bufs=2) as wtpool, \
             tc.tile_pool(name="sm", bufs=3) as smpool, \
             tc.tile_pool(name="fin", bufs=2) as fpool, \
             tc.tile_pool(name="ps1", bufs=4, space="PSUM") as ps1pool, \
             tc.tile_pool(name="ps2", bufs=2, space="PSUM") as ps2pool, \
             tc.tile_pool(name="ps3", bufs=1, space="PSUM") as ps3pool:

            # ---- constants ----
            wp_t = cpool.tile([C_IN, P * C_OUT], F16, tag="wp")
            nc.sync.dma_start(wp_t[:], w_in[:])
            bias_t = cpool.tile([C_OUT, 1], F32, tag="bias")
            nc.sync.dma_start(bias_t[:], bias_in[:])
            mask120_t = cpool.tile([128, 120], F16, tag="m120")
            nc.sync.dma_start(mask120_t[:], mask120_in[:])
            mask16_t = cpool.tile([128, 8], F16, tag="m16")
            nc.sync.dma_start(mask16_t[:], mask16_in[:])
            ones1_t = cpool.tile([1, 128], F16, tag="ones1")
            nc.sync.dma_start(ones1_t[:], ones1_in[:])
            kpb_t = cpool.tile([128, 48], F32, tag="kpb")
            nc.sync.dma_start(kpb_t[:], kpb_in[:])
            kpallf_t = cpool.tile([128, 45], F16, tag="kpallf")
            nc.sync.dma_start(kpallf_t[:], kpallf_in[:])
            onesc_t = cpool.tile([128, 1], F16, tag="onesc")
            nc.sync.dma_start(onesc_t[:], onesc_in[:])
            nidx_reg = nc.gpsimd.to_reg(GCHUNK)

            _main_pipeline(nc, tc, gpool, kwpool, kbpool, wtpool, smpool,
                           fpool, ps1pool, ps2pool, ps3pool, kp,
                           qrep_in, idx_in, out_t, table_in, wp_t, bias_t,
                           mask120_t, mask16_t, ones1_t, kpb_t,
                           onesc_t, nidx_reg, kpallf_t, dbg_t)
    return nc


def _main_pipeline(nc, tc, gpool, kwpool, kbpool, wtpool, smpool, fpool,
                   ps1pool, ps2pool, ps3pool, kp, qrep_in, idx_in, out_t,
                   table, wp_t, bias_t, mask120_t, mask16_t, ones1_t, kpb_t,
                   onesc_t, nidx_reg, kpallf_t=None, dbg_t=None):
    for kg in range(N_ST // KW_ST):  # kw group of 2 supertiles
        GQ = KW_ST * ST_Q            # 1024 queries
        GG = KW_ST * G_ST            # 128 g-cols
        NIDX_G = GQ * K              # 16384 idx per group
        gt = gpool.tile([128, GG, ROW16], F16, tag="gath")
        gt32 = gt[:].bitcast(F32)  # [128, GG, 64] f32 view
        # one idx load for the whole group
        idxg = smpool.tile([128, NIDX_G // 16], I16, tag="idxg")
        nc.sync.dma_start(
            idxg[:], idx_in[:, kg * (NIDX_G // 16):(kg + 1) * (NIDX_G // 16)])
        if "gather" in SKIP:
            nc.vector.memset(gt[:], 0.0)
        ncalls = NIDX_G // GCHUNK
        gcols = GG // ncalls
        for g in range(ncalls if "gather" not in SKIP else 0):
            nc.gpsimd.dma_gather(
                gt[:, g * gcols:(g + 1) * gcols, :], table[:],
                idxg[:, g * (GCHUNK // 16):(g + 1) * (GCHUNK // 16)],
                GCHUNK, nidx_reg, ROW16, queue_num=g % 4)
        # qrep slice
        qr = smpool.tile([128, GG, 3], F32, tag="qr")
        nc.sync.dma_start(qr[:], qrep_in[:, kg * GG:(kg + 1) * GG, :])
        # rel = s - q (fp16 out: costs ~1e-2 rel err end-to-end, gate 2e-2)
        rel = smpool.tile([128, GG, 3], F16, tag="rel")
        nc.vector.tensor_tensor(
            out=rel[:],
            in0=ap_view(gt32, 32, [[64, GG], [1, 3]]),
            in1=qr[:], op=mybir.AluOpType.subtract)
        # d2 batched: dall[g,p,d] = rel[g,d] - kp[p,d] (one TT), square
        # in place (one ACT), reduce over d (one DVE reduce) -> fp16 d2
        kwt16 = kwpool.tile([128, GG, P], F16, tag="kw16")
        dall = kwpool.tile([128, GG * P * 3], F16, tag="dall")
        if "kw" in SKIP:
            nc.vector.memset(kwt16[:], 0.0)
        if "kw" not in SKIP:
            nc.vector.tensor_tensor(
                out=dall[:],
                in0=ap_view(rel[:], 0, [[3, GG], [0, P], [1, 3]]),
                in1=ap_view(kpallf_t[:], 0, [[0, GG], [3, P], [1, 3]]),
                op=mybir.AluOpType.subtract)
            nc.scalar.activation(dall[:], dall[:],
                                 mybir.ActivationFunctionType.Square,
                                 bias=0.0, scale=1.0)
            with nc.allow_low_precision(
                    reason="fp16 sum of 3 squares; d2 needs ~1e-3 rel"):
                nc.vector.tensor_reduce(
                    out=ap_view(kwt16[:], 0, [[1, GG * P], [1, 1]]),
                    in_=ap_view(dall[:], 0, [[3, GG * P], [1, 3]]),
                    axis=mybir.AxisListType.X, op=mybir.AluOpType.add)
            # kw = relu(1 - sqrt(d2)/sigma), fp16 in place
            nc.scalar.activation(kwt16[:], kwt16[:],
                                 mybir.ActivationFunctionType.Sqrt,
                                 bias=0.0, scale=1.0)
            nc.scalar.activation(kwt16[:], kwt16[:],
                                 mybir.ActivationFunctionType.Relu,
                                 bias=1.0, scale=kpb_t[:, 46:47])
        if dbg_t is not None and kg == 6:
            stg = fpool.tile([128, 384], F32, tag="dbgstg")
            # z col per g
            nc.vector.tensor_copy(
                stg[:, 0:128],
                ap_view(gt[:], 70, [[ROW16, GG], [1, 1]]))
            # feat col 0 per g
            nc.vector.tensor_copy(
                stg[:, 128:256],
                ap_view(gt[:], 0, [[ROW16, GG], [1, 1]]))
            # kw p=0 per g
            nc.vector.tensor_copy(
                stg[:, 256:384],
                ap_view(kwt16[:], 0, [[P, GG], [1, 1]]))
            nc.sync.dma_start(dbg_t[:, 0:384], stg[:])

        for sti in range(KW_ST):
            st = kg * KW_ST + sti
            # kwbd (2 half-ST TT ops): [128, (bl32, q8, p15)] fp16
            kbd = kbpool.tile([128, 3840], F16, tag="kbd")
            kbd2 = kbpool.tile([128, 3840], F16, tag="kbd2")
            if "kwbd" in SKIP:
                nc.vector.memset(kbd[:], 0.0)
                nc.vector.memset(kbd2[:], 0.0)
            for hf, kb in ((0, kbd), (1, kbd2)) if "kwbd" not in SKIP else ():
                bl0 = sti * G_ST + hf * 32
                # p-major block layout: col (bl, p, q) so einsum2 rhs slices
                # are contiguous runs of 8
                nc.vector.tensor_tensor(
                    out=ap_view(kb[:], 0, [[120, 32], [8, 15], [1, 8]]),
                    in0=ap_view(kwt16[:], bl0 * P, [[P, 32], [1, P], [0, 8]]),
                    in1=ap_view(mask120_t[:], 0, [[0, 32], [8, 15], [1, 8]]),
                    op=mybir.AluOpType.mult)
            # einsum1: 64 blocks -> wtt fp16
            wtt = wtpool.tile([64, 7680], F16, tag="wt")
            if "e1" in SKIP:
                nc.vector.memset(wtt[:], 0.0)
            for bg in range(16 if "e1" not in SKIP else 0):
                pse1 = ps1pool.tile([64, 480], F32, tag="pse1")
                for j in range(4):
                    bl = bg * 4 + j          # block in supertile
                    blg = sti * G_ST + bl    # g-col in group tile
                    kb = kbd if bl < 32 else kbd2
                    kbl = bl % 32
                    nc.tensor.matmul(
                        pse1[:, j * 120:(j + 1) * 120],
                        ap_view(gt[:], blg * ROW16, [[1, C_IN]]),
                        ap_view(kb[:], kbl * 120, [[1, 120]]),
                        start=True, stop=True)
                # evict (split DVE/ACT), f32 -> fp16
                nc.vector.tensor_copy(
                    wtt[:, bg * 480:bg * 480 + 240], pse1[:, 0:240])
                nc.scalar.activation(
                    wtt[:, bg * 480 + 240:bg * 480 + 480], pse1[:, 240:480],
                    mybir.ActivationFunctionType.Copy, bias=0.0, scale=1.0)
            # count: zbd = z * mask16 (fp16) -> ones-col matmul -> replicate
            zbd = smpool.tile([128, 512], F16, tag="zbd")
            nc.vector.tensor_tensor(
                out=zbd[:].rearrange("a (g j q) -> a g j q", g=16, j=4),
                in0=ap_view(gt[:], (sti * G_ST) * ROW16 + 70,
                            [[512, 16], [128, 4], [0, 8]]),
                in1=ap_view(mask16_t[:], 0, [[0, 16], [0, 4], [1, 8]]),
                op=mybir.AluOpType.mult)
            pscnt = ps3pool.tile([1, 512], F32, tag="pscnt")
            nc.tensor.matmul(pscnt[:], onesc_t[:], zbd[:],
                             start=True, stop=True)
            cntrow = smpool.tile([1, 512], F16, tag="cntrow")
            nc.scalar.copy(cntrow[:], pscnt[:])
            psrep = ps3pool.tile([128, 512], F32, tag="psrep")
            nc.tensor.matmul(psrep[:], ones1_t[:], cntrow[:],
                             start=True, stop=True)
            cntinv = smpool.tile([128, 512], F32, tag="cntinv")
            nc.vector.tensor_scalar(out=cntinv[:], in0=psrep[:],
                                    scalar1=1.0, scalar2=None,
                                    op0=mybir.AluOpType.max)
            # 1/x = exp(-ln(x)); x is an integer count in [1, 16]
            nc.scalar.activation(cntinv[:], cntinv[:],
                                 mybir.ActivationFunctionType.Ln,
                                 bias=0.0, scale=1.0)
            nc.scalar.activation(cntinv[:], cntinv[:],
                                 mybir.ActivationFunctionType.Exp,
                                 bias=0.0, scale=-1.0)

            # einsum2: out[o, n] accumulated over p (fp16 operands)
            pse2 = ps2pool.tile([128, 512], F32, tag="pse2")
            for p in range(P if "e2" not in SKIP else 1):
                nc.tensor.matmul(
                    pse2[:],
                    ap_view(wp_t[:], p * C_OUT, [[1, C_OUT]]),
                    ap_view(wtt[:], p * 8, [[480, 16], [120, 4], [1, 8]]),
                    start=(p == 0), stop=True)
            if dbg_t is not None and st == 12:
                stg2 = fpool.tile([128, 1536], F32, tag="dbgstg2")
                nc.vector.tensor_copy(stg2[:, 0:512], cntinv[:])
                nc.vector.tensor_copy(stg2[:, 512:1024], pse2[:])
                nc.vector.tensor_copy(stg2[:, 1024:1536], zbd[:])
                nc.sync.dma_start(dbg_t[:, 512:2048], stg2[:])
                stg3 = fpool.tile([64, 512], F32, tag="dbgstg3")
                nc.vector.tensor_copy(stg3[:], wtt[:, 0:512])
                nc.sync.dma_start(dbg_t[0:64, 2048:2560], stg3[:])
            # divide by count, add bias, store transposed
            e2sb = fpool.tile([128, 512], F32, tag="e2sb")
            nc.vector.tensor_tensor(out=e2sb[:], in0=pse2[:], in1=cntinv[:],
                                    op=mybir.AluOpType.mult)
            nc.vector.tensor_scalar(out=e2sb[:], in0=e2sb[:],
                                    scalar1=bias_t[:], scalar2=None,
                                    op0=mybir.AluOpType.add)
            nc.sync.dma_start(out_t[:, st * 512:(st + 1) * 512], e2sb[:])


def _make_runner(nc, n_cores):
    bass2jax.install_neuronx_cc_hook()
    from jax.sharding import Mesh, PartitionSpec
    from jax.experimental.shard_map import shard_map

    partition_name = nc.partition_id_tensor.name if nc.partition_id_tensor else None
    in_names, out_names, out_avals, zero_outs = [], [], [], []
    for alloc in nc.m.functions[0].allocations:
        if not isinstance(alloc, mybir.MemoryLocationSet):
            continue
        name = alloc.memorylocations[0].name
        if alloc.kind == "ExternalInput":
            if name != partition_name:
                in_names.append(name)
        elif alloc.kind == "ExternalOutput":
            shape = tuple(alloc.tensor_shape)
            dtype = mybir.dt.np(alloc.dtype)
            out_names.append(name)
            out_avals.append(jax.core.ShapedArray(shape, dtype))
            zero_outs.append(np.zeros(shape, dtype))
    n_params = len(in_names)
    n_outs = len(out_avals)
    all_in = in_names + out_names + ([partition_name] if partition_name else [])

    def _body(*args):
        operands = list(args)
        if partition_name is not None:
            operands.append(bass2jax.partition_id_tensor())
        outs = bass2jax._bass_exec_p.bind(
            *operands, out_avals=tuple(out_avals), in_names=tuple(all_in),
            out_names=tuple(out_names), lowering_input_output_aliases=(),
            sim_require_finite=False, sim_require_nnan=False, nc=nc)
        return tuple(outs)

    devices = jax.devices()[:n_cores]
    mesh = Mesh(np.asarray(devices), ("core",))
    in_specs = (PartitionSpec("core"),) * (n_params + n_outs)
    out_specs = (PartitionSpec("core"),) * n_outs
    jit_fn = jax.jit(
        shard_map(_body, mesh=mesh, in_specs=in_specs, out_specs=out_specs,
                  check_rep=False), keep_unused=True)

    def run(in_maps):
        per_core = [[np.asarray(m[n]) for n in in_names] for m in in_maps]
        args = [np.concatenate([per_core[c][i] for c in range(n_cores)], axis=0)
                for i in range(n_params)]
        args += [np.zeros((n_cores * z.shape[0], *z.shape[1:]), z.dtype)
                 for z in zero_outs]
        outs = [np.asarray(o) for o in jit_fn(*args)]
        return [{n: outs[i].reshape(n_cores, *out_avals[i].shape)[c]
                 for i, n in enumerate(out_names)}
                for c in range(n_cores)], jit_fn, args

    return run


_BUILT = {}
_NCS = {}


def _get_runner(kp):
    key = kp.tobytes()
    if key not in _BUILT:
        nc = build_bass(kp)
        _NCS[key] = nc
        _BUILT[key] = _make_runner(nc, N_CORES)
    return _BUILT[key]


def _host_prep(query_points, support_points, support_features,
               neighbor_indices, weights, bias, kernel_points):
    qp = np.asarray(query_points, np.float32)
    sp = np.asarray(support_points, np.float32)
    sf = np.asarray(support_features, np.float32)
    ni = np.asarray(neighbor_indices)
    ni = np.clip(ni, 0, M - 1).astype(np.int16)
    w = np.ascontiguousarray(
        np.asarray(weights, np.float32).transpose(1, 0, 2).reshape(
            C_IN, P * C_OUT)).astype(np.float16)
    bias = np.asarray(bias, np.float32).reshape(C_OUT, 1)

    # host-built gather tables, one per batch
    tables = []
    for b in range(B):
        tbl = np.zeros((M, ROW16), np.float16)
        tbl[:, 0:C_IN] = sf[b].astype(np.float16)
        tblf = tbl.view(np.float32)
        tblf[:, 32:35] = sp[b]
        z = (np.abs(sf[b]).sum(axis=1) > 0).astype(np.float16)
        tbl[:, 70] = z
        tables.append(tbl)

    # p-major: col (p, q) -> 1 iff partition//16 == q
    mask120 = np.zeros((128, 120), np.float16)
    for q in range(8):
        for p in range(15):
            mask120[q * 16:(q + 1) * 16, p * 8 + q] = 1.0
    mask16 = np.zeros((128, 8), np.float16)
    for q in range(8):
        mask16[q * 16:(q + 1) * 16, q] = 1.0
    ones1 = np.ones((1, 128), np.float16)
    kpv = np.asarray(kernel_points, np.float32)
    kpb = np.zeros((128, 48), np.float32)
    for p in range(P):
        for d in range(3):
            kpb[:, 3 * p + d] = -kpv[p, d]
    kpb[:, 45] = 1e-10
    kpb[:, 46] = -1.0 / SIGMA
    kpallf = np.tile(kpv.reshape(1, 45), (128, 1)).astype(np.float16)

    in_maps = []
    for c in range(N_CORES):
        b, half = divmod(c, 2)
        n0 = half * NQ_CORE
        idx = ni[b, n0:n0 + NQ_CORE, :].reshape(NK_CORE)
        # chunk order: idx j in chunk -> partition j%16 (k), col j//16;
        # stream order is already (query, k) = natural
        idx_l = idx.reshape(NK_CORE // 16, 16).T          # [16, NK/16]
        idx_l = np.tile(idx_l, (8, 1))                    # [128, NK/16]
        qrep = np.repeat(qp[b, n0:n0 + NQ_CORE, :], K, axis=0)  # [NK, 3]
        qrep = qrep.reshape(NK_CORE // 128, 128, 3).transpose(1, 0, 2)
        qrep = np.ascontiguousarray(qrep)
        in_maps.append({
            "table": tables[b], "qrep": qrep,
            "idx": np.ascontiguousarray(idx_l),
            "w": w, "bias": bias, "mask120": mask120, "mask16": mask16,
            "ones1": ones1, "kpb": kpb, "kpallf": kpallf,
            "onesc": np.ones((128, 1), np.float16),
        })
    return in_maps


def kernel(query_points, support_points, support_features, neighbor_indices,
           weights, bias, kernel_points):
    kp = np.asarray(kernel_points, np.float32)
    run = _get_runner(kp)
    in_maps = _host_prep(query_points, support_points, support_features,
                         neighbor_indices, weights, bias, kernel_points)
    results, _, _ = run(in_maps)
    out = np.zeros((B, N, C_OUT), np.float32)
    for c in range(N_CORES):
        b, half = divmod(c, 2)
        n0 = half * NQ_CORE
        out[b, n0:n0 + NQ_CORE, :] = results[c]["out"].T
    return out


# revision 7
# speedup vs baseline: 1.2678x; 1.0295x over previous
"""KPConv (nn_KPConvFPN) Trainium2 Bass kernel, v2.

Sharding: 8 cores; core c handles batch b=c//2, query half (c%2)*8192.
Changes vs v1:
  - Combined gather table is built on HOST (numpy) and shipped as input:
    row m (256B) = [64 x fp16 feat | f32 sx,sy,sz at f32-cols 32..34 |
    fp16 z at col 70], z = (sum_c |f| > 0).
  - All matmuls fp16 (weights, kbd, wtt, count) -> 1 cy/row on PE.
  - einsum2 output [o=128, n=512] divided/biased in place and stored to a
    TRANSPOSED dram output [C_OUT, NQ]; host transposes back (no PE
    transposes, no trsb copies).
  - count path: replicate-then-reciprocal on [128,512] (fast) instead of
    reciprocal on [1,512].
  - One idx DMA per kw-group instead of 16 small ones.
"""
import json
import math
import os

SKIP = set()

import numpy as np
import jax

import concourse.bass as bass
import concourse.mybir as mybir
from concourse.tile import TileContext
from concourse import library_config
from concourse import bass2jax

F32 = mybir.dt.float32
F16 = mybir.dt.float16
I16 = mybir.dt.int16

B, N, M, K = 4, 16384, 16384, 16
C_IN, C_OUT, P = 64, 128, 15
SIGMA = 0.03
N_CORES = 8
NQ_CORE = N // 2            # 8192 queries per core
NK_CORE = NQ_CORE * K       # 131072 gathered rows per core
ST_Q = 512                  # queries per supertile
N_ST = NQ_CORE // ST_Q      # 16
KW_ST = 2                   # supertiles per kw group
G_ST = ST_Q * K // 128      # 64 g-cols per supertile
ROW16 = 128                 # fp16 units per table row (256B)
GCHUNK = 1024               # idx per dma_gather call

# ---------------------------------------------------------------------------
# walrus workaround: this nix walrus build supports ONE sync-wait per
# instruction; split extra waits onto NoOps inserted before the offender
# (same-engine program order preserves semantics). Also run
# codegen_inst_isa_subclasses (Bacc does; raw Bass doesn't) so extended
# instructions get their ISA bytes.
_orig_to_json_bytes = bass.Bass.to_json_bytes


def _fix_block(bb, ctr):
    insts = bb.get("instructions")
    if not isinstance(insts, list):
        return
    new = []
    for inst in insts:
        si = inst.get("sync_info")
        ow = si.get("on_wait") if isinstance(si, dict) else None
        if ow and len(ow) > 1:
            for w in ow[:-1]:
                ctr[0] += 1
                nop = {"engine": inst["engine"], "ins": [], "outs": [],
                       "name": f"I-wsplit-{ctr[0]}", "opcode": "NoOp",
                       "sync_info": {"on_update": [], "on_wait": [w]},
                       "text_hint": "wsplit"}
                if "debug" in inst:
                    nop["debug"] = inst["debug"]
                new.append(nop)
            si["on_wait"] = [ow[-1]]
        new.append(inst)
    bb["instructions"] = new


def _walk(o, ctr):
    if isinstance(o, dict):
        if isinstance(o.get("instructions"), list):
            _fix_block(o, ctr)
        for v in o.values():
            _walk(v, ctr)
    elif isinstance(o, list):
        for v in o:
            _walk(v, ctr)


def _to_json_bytes_split(self):
    mybir.codegen_inst_isa_subclasses(self)
    raw = _orig_to_json_bytes(self)
    d = json.loads(raw)
    ctr = [0]
    _walk(d, ctr)
    return json.dumps(d).encode()


bass.Bass.to_json_bytes = _to_json_bytes_split


def ap_view(t_ap, extra_offset, dims):
    """AP over tile t_ap with explicit free dims [[step, count], ...]
    (steps in elements); partition dim is taken from the tile."""
    return bass.AP(t_ap.tensor, t_ap.offset + extra_offset,
                   [t_ap.ap[0]] + list(dims))


DEBUG = False


def build_bass(kp, skip=()):
    global SKIP
    SKIP = set(skip)
    """kp: (15, 3) float32 numpy kernel points (runtime values baked)."""
    nc = bass.Bass(dynamic_dma_scratch_size=32768, num_swdge_queues=4)

    table_in = nc.dram_tensor("table", [M, ROW16], F16, kind="ExternalInput")
    qrep_in = nc.dram_tensor("qrep", [128, NK_CORE // 128, 3], F32,
                             kind="ExternalInput")
    idx_in = nc.dram_tensor("idx", [128, NK_CORE // 16], I16,
                            kind="ExternalInput")
    w_in = nc.dram_tensor("w", [C_IN, P * C_OUT], F16, kind="ExternalInput")
    bias_in = nc.dram_tensor("bias", [C_OUT, 1], F32, kind="ExternalInput")
    mask120_in = nc.dram_tensor("mask120", [128, 120], F16, kind="ExternalInput")
    mask16_in = nc.dram_tensor("mask16", [128, 8], F16, kind="ExternalInput")
    ones1_in = nc.dram_tensor("ones1", [1, 128], F16, kind="ExternalInput")
    kpb_in = nc.dram_tensor("kpb", [128, 48], F32, kind="ExternalInput")
    kpallf_in = nc.dram_tensor("kpallf", [128, 45], F16, kind="ExternalInput")
    onesc_in = nc.dram_tensor("onesc", [128, 1], F16, kind="ExternalInput")
    out_t = nc.dram_tensor("out", [C_OUT, NQ_CORE], F32, kind="ExternalOutput")
    dbg_t = (nc.dram_tensor("dbg", [128, 4096], F32, kind="ExternalOutput")
             if DEBUG else None)

    # library load as raw preamble (before Tile scheduling) so it is
    # guaranteed to precede every dma_gather on the Pool engine.
    nc.gpsimd.load_library(library_config.mlp)

    with TileContext(nc) as tc:
        with tc.tile_pool(name="const", bufs=1) as cpool, \
             tc.tile_pool(name="gath", bufs=2) as gpool, \
             tc.tile_pool(name="kwp", bufs=2) as kwpool, \
             tc.tile_pool(name="kbd", bufs=2) as kbpool, \
             tc.tile_pool(name="wt", bufs=2) as wtpool, \
             tc.tile_pool(name="sm", bufs=2) as smpool, \
             tc.tile_pool(name="fin", bufs=2) as fpool, \
             tc.tile_pool(name="ps1", bufs=4, space="PSUM") as ps1pool, \
             tc.tile_pool(name="ps2", bufs=2, space="PSUM") as ps2pool, \
             tc.tile_pool(name="ps3", bufs=1, space="PSUM") as ps3pool:

            # ---- constants ----
            wp_t = cpool.tile([C_IN, P * C_OUT], F16, tag="wp")
            nc.sync.dma_start(wp_t[:], w_in[:])
            bias_t = cpool.tile([C_OUT, 1], F32, tag="bias")
            nc.sync.dma_start(bias_t[:], bias_in[:])
            mask120_t = cpool.tile([128, 120], F16, tag="m120")
            nc.sync.dma_start(mask120_t[:], mask120_in[:])
            mask16_t = cpool.tile([128, 8], F16, tag="m16")
            nc.sync.dma_start(mask16_t[:], mask16_in[:])
            ones1_t = cpool.tile([1, 128], F16, tag="ones1")
            nc.sync.dma_start(ones1_t[:], ones1_in[:])
            kpb_t = cpool.tile([128, 48], F32, tag="kpb")
            nc.sync.dma_start(kpb_t[:], kpb_in[:])
            kpallf_t = cpool.tile([128, 45], F16, tag="kpallf")
            nc.sync.dma_start(kpallf_t[:], kpallf_in[:])
            onesc_t = cpool.tile([128, 1], F16, tag="onesc")
            nc.sync.dma_start(onesc_t[:], onesc_in[:])
            nidx_reg = nc.gpsimd.to_reg(GCHUNK)

            _main_pipeline(nc, tc, gpool, kwpool, kbpool, wtpool, smpool,
                           fpool, ps1pool, ps2pool, ps3pool, kp,
                           qrep_in, idx_in, out_t, table_in, wp_t, bias_t,
                           mask120_t, mask16_t, ones1_t, kpb_t,
                           onesc_t, nidx_reg, kpallf_t, dbg_t)
    return nc


def _main_pipeline(nc, tc, gpool, kwpool, kbpool, wtpool, smpool, fpool,
                   ps1pool, ps2pool, ps3pool, kp, qrep_in, idx_in, out_t,
                   table, wp_t, bias_t, mask120_t, mask16_t, ones1_t, kpb_t,
                   onesc_t, nidx_reg, kpallf_t=None, dbg_t=None):
    for kg in range(N_ST // KW_ST):  # kw group of 2 supertiles
        GQ = KW_ST * ST_Q            # 1024 queries
        GG = KW_ST * G_ST            # 128 g-cols
        NIDX_G = GQ * K              # 16384 idx per group
        gt = gpool.tile([128, GG, ROW16], F16, tag="gath")
        gt32 = gt[:].bitcast(F32)  # [128, GG, 64] f32 view
        # one idx load for the whole group
        idxg = smpool.tile([128, NIDX_G // 16], I16, tag="idxg")
        nc.sync.dma_start(
            idxg[:], idx_in[:, kg * (NIDX_G // 16):(kg + 1) * (NIDX_G // 16)])
        if "gather" in SKIP:
            nc.vector.memset(gt[:], 0.0)
        ncalls = NIDX_G // GCHUNK
        gcols = GG // ncalls
        for g in range(ncalls if "gather" not in SKIP else 0):
            nc.gpsimd.dma_gather(
                gt[:, g * gcols:(g + 1) * gcols, :], table[:],
                idxg[:, g * (GCHUNK // 16):(g + 1) * (GCHUNK // 16)],
                GCHUNK, nidx_reg, ROW16, queue_num=g % 4)
        # qrep slice
        qr = smpool.tile([128, GG, 3], F32, tag="qr")
        nc.sync.dma_start(qr[:], qrep_in[:, kg * GG:(kg + 1) * GG, :])
        # rel = s - q (fp16 out: costs ~1e-2 rel err end-to-end, gate 2e-2)
        rel = smpool.tile([128, GG, 3], F16, tag="rel")
        nc.vector.tensor_tensor(
            out=rel[:],
            in0=ap_view(gt32, 32, [[64, GG], [1, 3]]),
            in1=qr[:], op=mybir.AluOpType.subtract)
        # d2 batched: dall[g,p,d] = rel[g,d] - kp[p,d] (one TT), square
        # in place (one ACT), reduce over d (one DVE reduce) -> fp16 d2
        kwt16 = kwpool.tile([128, GG, P], F16, tag="kw16")
        dall = kwpool.tile([128, GG * P * 3], F16, tag="dall")
        if "kw" in SKIP:
            nc.vector.memset(kwt16[:], 0.0)
        if "kw" not in SKIP:
            nc.vector.tensor_tensor(
                out=dall[:],
                in0=ap_view(rel[:], 0, [[3, GG], [0, P], [1, 3]]),
                in1=ap_view(kpallf_t[:], 0, [[0, GG], [3, P], [1, 3]]),
                op=mybir.AluOpType.subtract)
            nc.scalar.activation(dall[:], dall[:],
                                 mybir.ActivationFunctionType.Square,
                                 bias=0.0, scale=1.0)
            with nc.allow_low_precision(
                    reason="fp16 sum of 3 squares; d2 needs ~1e-3 rel"):
                nc.vector.tensor_reduce(
                    out=ap_view(kwt16[:], 0, [[1, GG * P], [1, 1]]),
                    in_=ap_view(dall[:], 0, [[3, GG * P], [1, 3]]),
                    axis=mybir.AxisListType.X, op=mybir.AluOpType.add)
            # kw = relu(1 - sqrt(d2)/sigma), fp16 in place
            nc.scalar.activation(kwt16[:], kwt16[:],
                                 mybir.ActivationFunctionType.Sqrt,
                                 bias=0.0, scale=1.0)
            nc.scalar.activation(kwt16[:], kwt16[:],
                                 mybir.ActivationFunctionType.Relu,
                                 bias=1.0, scale=kpb_t[:, 46:47])
        if dbg_t is not None and kg == 6:
            stg = fpool.tile([128, 384], F32, tag="dbgstg")
            # z col per g
            nc.vector.tensor_copy(
                stg[:, 0:128],
                ap_view(gt[:], 70, [[ROW16, GG], [1, 1]]))
            # feat col 0 per g
            nc.vector.tensor_copy(
                stg[:, 128:256],
                ap_view(gt[:], 0, [[ROW16, GG], [1, 1]]))
            # kw p=0 per g
            nc.vector.tensor_copy(
                stg[:, 256:384],
                ap_view(kwt16[:], 0, [[P, GG], [1, 1]]))
            nc.sync.dma_start(dbg_t[:, 0:384], stg[:])

        for sti in range(KW_ST):
            st = kg * KW_ST + sti
            # kwbd (2 half-ST TT ops): [128, (bl32, q8, p15)] fp16
            kbd = kbpool.tile([128, 3840], F16, tag="kbd")
            kbd2 = kbpool.tile([128, 3840], F16, tag="kbd2")
            if "kwbd" in SKIP:
                nc.vector.memset(kbd[:], 0.0)
                nc.vector.memset(kbd2[:], 0.0)
            for hf, kb in ((0, kbd), (1, kbd2)) if "kwbd" not in SKIP else ():
                bl0 = sti * G_ST + hf * 32
                # p-major block layout: col (bl, p, q) so einsum2 rhs slices
                # are contiguous runs of 8
                nc.vector.tensor_tensor(
                    out=ap_view(kb[:], 0, [[120, 32], [8, 15], [1, 8]]),
                    in0=ap_view(kwt16[:], bl0 * P, [[P, 32], [1, P], [0, 8]]),
                    in1=ap_view(mask120_t[:], 0, [[0, 32], [8, 15], [1, 8]]),
                    op=mybir.AluOpType.mult)
            # einsum1: 64 blocks -> wtt fp16
            wtt = wtpool.tile([64, 7680], F16, tag="wt")
            if "e1" in SKIP:
                nc.vector.memset(wtt[:], 0.0)
            for bg in range(16 if "e1" not in SKIP else 0):
                pse1 = ps1pool.tile([64, 480], F32, tag="pse1")
                for j in range(4):
                    bl = bg * 4 + j          # block in supertile
                    blg = sti * G_ST + bl    # g-col in group tile
                    kb = kbd if bl < 32 else kbd2
                    kbl = bl % 32
                    nc.tensor.matmul(
                        pse1[:, j * 120:(j + 1) * 120],
                        ap_view(gt[:], blg * ROW16, [[1, C_IN]]),
                        ap_view(kb[:], kbl * 120, [[1, 120]]),
                        start=True, stop=True)
                # evict (split DVE/ACT), f32 -> fp16
                nc.vector.tensor_copy(
                    wtt[:, bg * 480:bg * 480 + 240], pse1[:, 0:240])
                nc.scalar.activation(
                    wtt[:, bg * 480 + 240:bg * 480 + 480], pse1[:, 240:480],
                    mybir.ActivationFunctionType.Copy, bias=0.0, scale=1.0)
            # count: zbd = z * mask16 (fp16) -> ones-col matmul -> replicate
            zbd = smpool.tile([128, 512], F16, tag="zbd")
            nc.vector.tensor_tensor(
                out=zbd[:].rearrange("a (g j q) -> a g j q", g=16, j=4),
                in0=ap_view(gt[:], (sti * G_ST) * ROW16 + 70,
                            [[512, 16], [128, 4], [0, 8]]),
                in1=ap_view(mask16_t[:], 0, [[0, 16], [0, 4], [1, 8]]),
                op=mybir.AluOpType.mult)
            pscnt = ps3pool.tile([1, 512], F32, tag="pscnt")
            nc.tensor.matmul(pscnt[:], onesc_t[:], zbd[:],
                             start=True, stop=True)
            cntrow = smpool.tile([1, 512], F16, tag="cntrow")
            nc.scalar.copy(cntrow[:], pscnt[:])
            psrep = ps3pool.tile([128, 512], F32, tag="psrep")
            nc.tensor.matmul(psrep[:], ones1_t[:], cntrow[:],
                             start=True, stop=True)
            cntinv = smpool.tile([128, 512], F32, tag="cntinv")
            nc.vector.tensor_scalar(out=cntinv[:], in0=psrep[:],
                                    scalar1=1.0, scalar2=None,
                                    op0=mybir.AluOpType.max)
            # 1/x = exp(-ln(x)); x is an integer count in [1, 16]
            nc.scalar.activation(cntinv[:], cntinv[:],
                                 mybir.ActivationFunctionType.Ln,
                                 bias=0.0, scale=1.0)
            nc.scalar.activation(cntinv[:], cntinv[:],
                                 mybir.ActivationFunctionType.Exp,
                                 bias=0.0, scale=-1.0)

            # einsum2: out[o, n] accumulated over p (fp16 operands)
            pse2 = ps2pool.tile([128, 512], F32, tag="pse2")
            for p in range(P if "e2" not in SKIP else 1):
                nc.tensor.matmul(
                    pse2[:],
                    ap_view(wp_t[:], p * C_OUT, [[1, C_OUT]]),
                    ap_view(wtt[:], p * 8, [[480, 16], [120, 4], [1, 8]]),
                    start=(p == 0), stop=True)
            if dbg_t is not None and st == 12:
                stg2 = fpool.tile([128, 1536], F32, tag="dbgstg2")
                nc.vector.tensor_copy(stg2[:, 0:512], cntinv[:])
                nc.vector.tensor_copy(stg2[:, 512:1024], pse2[:])
                nc.vector.tensor_copy(stg2[:, 1024:1536], zbd[:])
                nc.sync.dma_start(dbg_t[:, 512:2048], stg2[:])
                stg3 = fpool.tile([64, 512], F32, tag="dbgstg3")
                nc.vector.tensor_copy(stg3[:], wtt[:, 0:512])
                nc.sync.dma_start(dbg_t[0:64, 2048:2560], stg3[:])
            # divide by count, add bias, store transposed
            e2sb = fpool.tile([128, 512], F32, tag="e2sb")
            nc.vector.tensor_tensor(out=e2sb[:], in0=pse2[:], in1=cntinv[:],
                                    op=mybir.AluOpType.mult)
            nc.vector.tensor_scalar(out=e2sb[:], in0=e2sb[:],
                                    scalar1=bias_t[:], scalar2=None,
                                    op0=mybir.AluOpType.add)
            nc.sync.dma_start(out_t[:, st * 512:(st + 1) * 512], e2sb[:])


def _make_runner(nc, n_cores):
    bass2jax.install_neuronx_cc_hook()
    from jax.sharding import Mesh, PartitionSpec
    from jax.experimental.shard_map import shard_map

    partition_name = nc.partition_id_tensor.name if nc.partition_id_tensor else None
    in_names, out_names, out_avals, zero_outs = [], [], [], []
    for alloc in nc.m.functions[0].allocations:
        if not isinstance(alloc, mybir.MemoryLocationSet):
            continue
        name = alloc.memorylocations[0].name
        if alloc.kind == "ExternalInput":
            if name != partition_name:
                in_names.append(name)
        elif alloc.kind == "ExternalOutput":
            shape = tuple(alloc.tensor_shape)
            dtype = mybir.dt.np(alloc.dtype)
            out_names.append(name)
            out_avals.append(jax.core.ShapedArray(shape, dtype))
            zero_outs.append(np.zeros(shape, dtype))
    n_params = len(in_names)
    n_outs = len(out_avals)
    all_in = in_names + out_names + ([partition_name] if partition_name else [])

    def _body(*args):
        operands = list(args)
        if partition_name is not None:
            operands.append(bass2jax.partition_id_tensor())
        outs = bass2jax._bass_exec_p.bind(
            *operands, out_avals=tuple(out_avals), in_names=tuple(all_in),
            out_names=tuple(out_names), lowering_input_output_aliases=(),
            sim_require_finite=False, sim_require_nnan=False, nc=nc)
        return tuple(outs)

    devices = jax.devices()[:n_cores]
    mesh = Mesh(np.asarray(devices), ("core",))
    in_specs = (PartitionSpec("core"),) * (n_params + n_outs)
    out_specs = (PartitionSpec("core"),) * n_outs
    jit_fn = jax.jit(
        shard_map(_body, mesh=mesh, in_specs=in_specs, out_specs=out_specs,
                  check_rep=False), keep_unused=True)

    def run(in_maps):
        per_core = [[np.asarray(m[n]) for n in in_names] for m in in_maps]
        args = [np.concatenate([per_core[c][i] for c in range(n_cores)], axis=0)
                for i in range(n_params)]
        args += [np.zeros((n_cores * z.shape[0], *z.shape[1:]), z.dtype)
                 for z in zero_outs]
        outs = [np.asarray(o) for o in jit_fn(*args)]
        return [{n: outs[i].reshape(n_cores, *out_avals[i].shape)[c]
                 for i, n in enumerate(out_names)}
                for c in range(n_cores)], jit_fn, args

    return run


_BUILT = {}
_NCS = {}


def _get_runner(kp):
    key = kp.tobytes()
    if key not in _BUILT:
        nc = build_bass(kp)
        _NCS[key] = nc
        _BUILT[key] = _make_runner(nc, N_CORES)
    return _BUILT[key]


def _host_prep(query_points, support_points, support_features,
               neighbor_indices, weights, bias, kernel_points):
    qp = np.asarray(query_points, np.float32)
    sp = np.asarray(support_points, np.float32)
    sf = np.asarray(support_features, np.float32)
    ni = np.asarray(neighbor_indices)
    ni = np.clip(ni, 0, M - 1).astype(np.int16)
    w = np.ascontiguousarray(
        np.asarray(weights, np.float32).transpose(1, 0, 2).reshape(
            C_IN, P * C_OUT)).astype(np.float16)
    bias = np.asarray(bias, np.float32).reshape(C_OUT, 1)

    # host-built gather tables, one per batch
    tables = []
    for b in range(B):
        tbl = np.zeros((M, ROW16), np.float16)
        tbl[:, 0:C_IN] = sf[b].astype(np.float16)
        tblf = tbl.view(np.float32)
        tblf[:, 32:35] = sp[b]
        z = (np.abs(sf[b]).sum(axis=1) > 0).astype(np.float16)
        tbl[:, 70] = z
        tables.append(tbl)

    # p-major: col (p, q) -> 1 iff partition//16 == q
    mask120 = np.zeros((128, 120), np.float16)
    for q in range(8):
        for p in range(15):
            mask120[q * 16:(q + 1) * 16, p * 8 + q] = 1.0
    mask16 = np.zeros((128, 8), np.float16)
    for q in range(8):
        mask16[q * 16:(q + 1) * 16, q] = 1.0
    ones1 = np.ones((1, 128), np.float16)
    kpv = np.asarray(kernel_points, np.float32)
    kpb = np.zeros((128, 48), np.float32)
    for p in range(P):
        for d in range(3):
            kpb[:, 3 * p + d] = -kpv[p, d]
    kpb[:, 45] = 1e-10
    kpb[:, 46] = -1.0 / SIGMA
    kpallf = np.tile(kpv.reshape(1, 45), (128, 1)).astype(np.float16)

    in_maps = []
    for c in range(N_CORES):
        b, half = divmod(c, 2)
        n0 = half * NQ_CORE
        idx = ni[b, n0:n0 + NQ_CORE, :].reshape(NK_CORE)
        # chunk order: idx j in chunk -> partition j%16 (k), col j//16;
        # stream order is already (query, k) = natural
        idx_l = idx.reshape(NK_CORE // 16, 16).T          # [16, NK/16]
        idx_l = np.tile(idx_l, (8, 1))                    # [128, NK/16]
        qrep = np.repeat(qp[b, n0:n0 + NQ_CORE, :], K, axis=0)  # [NK, 3]
        qrep = qrep.reshape(NK_CORE // 128, 128, 3).transpose(1, 0, 2)
        qrep = np.ascontiguousarray(qrep)
        in_maps.append({
            "table": tables[b], "qrep": qrep,
            "idx": np.ascontiguousarray(idx_l),
            "w": w, "bias": bias, "mask120": mask120, "mask16": mask16,
            "ones1": ones1, "kpb": kpb, "kpallf": kpallf,
            "onesc": np.ones((128, 1), np.float16),
        })
    return in_maps


def kernel(query_points, support_points, support_features, neighbor_indices,
           weights, bias, kernel_points):
    kp = np.asarray(kernel_points, np.float32)
    run = _get_runner(kp)
    in_maps = _host_prep(query_points, support_points, support_features,
                         neighbor_indices, weights, bias, kernel_points)
    results, _, _ = run(in_maps)
    out = np.zeros((B, N, C_OUT), np.float32)
    for c in range(N_CORES):
        b, half = divmod(c, 2)
        n0 = half * NQ_CORE
        out[b, n0:n0 + NQ_CORE, :] = results[c]["out"].T
    return out
